# revision 22
# baseline (speedup 1.0000x reference)
"""Trainium2 Bass kernel for the MANN network (LSTM scan + memory-write scan).

Self-contained: hardcodes all shapes. kernel(**inputs) takes full numpy inputs
and returns the full [128, 40] final memory matrix.

Structure (single-core program, replicated on 8 cores via SPMD):
  Phase A (batch): GXT[p, jj, t] = gate pre-activations from x/y (PE matmuls -> DRAM)
  Loop over 32 chunks of 128 steps:
    B(c): 128 sequential LSTM steps (tanh-only nonlinearities, fp16 W_hh stationary)
    C(c): chunk keys/sigma/kn batch matmuls
    D(c-1): 128 sequential memory-update steps (PE outer products, Newton rsqrt,
            softmax normalization folded lazily into the next write weights)
"""

import sys

import numpy as np

# concourse (Bass) lives in the TRN RL repo; make it importable regardless of cwd
for _p in ("/opt/trn_rl_repo", "/root/.axon_site/_ro/trn_rl_repo"):
    try:
        import concourse  # noqa: F401
        break
    except ImportError:
        if _p not in sys.path:
            sys.path.insert(0, _p)

T, D, F, H, NS, KD = 4096, 512, 256, 200, 128, 40
TC = 128                  # steps per chunk
NCH = T // TC             # 32 chunks
G4P = 1024                # padded gate vector (4 gates x 256)
QUAKE_F = 1597463007.0    # 0x5f3759df as float
N_CORES = 8


# ---------------------------------------------------------------- host prep --
def _prep(inputs):
    f32 = np.float32
    x = np.ascontiguousarray(inputs["x_train"], f32)
    y = np.ascontiguousarray(inputs["y_train"], f32)
    W_in = np.asarray(inputs["W_in"], f32)
    b_in = np.asarray(inputs["b_in"], f32)
    W_ih = np.asarray(inputs["W_ih"], f32)
    W_hh = np.asarray(inputs["W_hh"], f32)
    b_ih = np.asarray(inputs["b_ih"], f32)
    b_hh = np.asarray(inputs["b_hh"], f32)
    W_k = np.asarray(inputs["W_k"], f32)
    b_k = np.asarray(inputs["b_k"], f32)
    W_s = np.asarray(inputs["W_s"], f32)
    b_s = np.asarray(inputs["b_s"], f32)

    # Gate reorder (i, f, gg, o) -> (i, f, o, gg); sigmoid gates scaled by 0.5
    # (sigmoid(v) = 0.5*tanh(0.5 v)+0.5), pad each gate 200 -> 256 rows.
    gate_src = [0, 1, 3, 2]
    scale = [0.5, 0.5, 0.5, 1.0]
    b_tot = b_ih + b_hh
    Wtil = np.zeros((G4P, F + 2), f32)   # cols 0:256 = x feats, 256 = y, 257 = bias
    Whhp = np.zeros((G4P, H), f32)
    for g in range(4):
        src = gate_src[g]
        rows = slice(256 * g, 256 * g + H)
        Wtil[rows, 0:F + 1] = scale[g] * W_ih[200 * src:200 * src + H, :]
        Wtil[rows, F + 1] = scale[g] * b_tot[200 * src:200 * src + H]
        Whhp[rows, :] = scale[g] * W_hh[200 * src:200 * src + H, :]

    watil_t = np.ascontiguousarray(Wtil.T)                     # [258, 1024] f32
    whhT = np.ascontiguousarray(Whhp.T).astype(np.float16)     # [200, 1024] fp16
    # keys/sigma weights: [201, 41]: rows 0:200 hid, row 200 bias; col 40 scaled W_s
    wks = np.zeros((225, KD + 1), f32)   # 0:200 hid, 200:224 zero pad, 224 bias
    wks[0:H, 0:KD] = W_k
    wks[224, 0:KD] = b_k
    wks[0:H, KD] = 0.5 * W_s[:, 0]
    wks[224, KD] = 0.5 * b_s[0]
    wks = wks.astype(np.float16)

    ysh1 = np.zeros((2, T), f32)          # row0 = y_shift, row1 = ones
    ysh1[0, 1:] = y[:-1, 0]
    ysh1[1, :] = 1.0
    return {
        "x_train": x,
        "ysh1": ysh1,
        "watil_t": watil_t,
        "whht": whhT,
        "wks": wks,
        "w_in": np.ascontiguousarray(W_in),
        "b_in": np.ascontiguousarray(b_in.reshape(2, 128)),   # [m, p] -> load as [128,2] via rearrange
    }


# ------------------------------------------------------------- bass program --
def build(nc, tc):
    import concourse.bass as bass
    from concourse import mybir
    from concourse.bass import ds

    f32 = mybir.dt.float32
    f16 = mybir.dt.float16
    u32 = mybir.dt.uint32
    AF = mybir.ActivationFunctionType
    OP = mybir.AluOpType
    X = mybir.AxisListType.X

    x_d = nc.dram_tensor("x_train", [T, D], f32, kind="ExternalInput")
    y_d = nc.dram_tensor("ysh1", [2, T], f32, kind="ExternalInput")
    watil_d = nc.dram_tensor("watil_t", [F + 2, G4P], f32, kind="ExternalInput")
    whht_d = nc.dram_tensor("whht", [H, G4P], f16, kind="ExternalInput")
    wks_d = nc.dram_tensor("wks", [225, KD + 1], f16, kind="ExternalInput")
    win_d = nc.dram_tensor("w_in", [D, F], f32, kind="ExternalInput")
    bin_d = nc.dram_tensor("b_in", [2, 128], f32, kind="ExternalInput")
    m_out = nc.dram_tensor("m_out", [NS, KD], f32, kind="ExternalOutput")
    gxt_d = nc.dram_tensor("gxt", [128, 8, T + TC], f32)  # internal scratch

    from contextlib import ExitStack
    stack = ExitStack()

    singles = stack.enter_context(tc.tile_pool(name="singles", bufs=1))

    # ---------------- persistent loop tiles ----------------
    whh_lo = singles.tile([128, G4P], f16)
    whh_hi = singles.tile([72, G4P], f16)
    wks_lo = singles.tile([128, KD + 1], f16)
    wks_hi = singles.tile([97, KD + 1], f16)
    ident128 = singles.tile([128, 128], f32)
    ident40 = singles.tile([40, 40], f32)
    ones40 = singles.tile([40, 1], f32)
    ones128c = singles.tile([128, 1], f32)

    h16 = singles.tile([128, 2], f16)       # hidden state (col0 = h[0:128], col1 = h[128:200]+pad)
    tg = singles.tile([128, 10], f32)       # cols 0:8 tanh(gates); cols 8:10 = c state
    sg6 = singles.tile([128, 6], f32)       # sigmoid(i,f,o)
    gsum = singles.tile([128, 8], f32)      # gates pre-activation (gx + Whh h)
    thc = singles.tile([128, 2], f32)

    Mt = singles.tile([40, 128], f32)       # memory, transposed [key, slot]
    Mt2 = singles.tile([40, 128], f32)
    rn = singles.tile([128, 1], f32)        # rsqrt(row-norm^2) newton state
    rn2 = singles.tile([128, 1], f32)
    n2c = singles.tile([128, 1], f32)
    nt2 = singles.tile([128, 1], f32)
    nt3 = singles.tile([128, 1], f32)
    qu1 = singles.tile([128, 1], u32)
    qf1 = singles.tile([128, 1], f32)
    qf2 = singles.tile([128, 1], f32)
    qy0 = singles.tile([128, 1], u32)
    uvec = singles.tile([128, 1], f32)
    tht = singles.tile([128, 1], f32)
    ea = singles.tile([128, 1], f32)
    eb = singles.tile([128, 1], f32)
    e_col = singles.tile([128, 1], f32)
    ww_row = singles.tile([1, 128], f32)
    rs = singles.tile([1, 1], f32)
    sigrs = singles.tile([1, 1], f32)
    m_sb = singles.tile([NS, KD], f32)

    # ping-pong chunk tiles
    gx_tile = [singles.tile([128, 8, TC], f32, tag=f"gx{p}", name=f"gx{p}") for p in range(2)]
    hidc_a = [singles.tile([128, TC], f16, tag=f"ha{p}", name=f"ha{p}") for p in range(2)]
    hidc_b = [singles.tile([97, TC], f16, tag=f"hb{p}", name=f"hb{p}") for p in range(2)]
    keysc = [singles.tile([128, KD + 1], f32, tag=f"kc{p}", name=f"kc{p}") for p in range(2)]
    knc = [singles.tile([40, 128], f32, tag=f"kn{p}", name=f"kn{p}") for p in range(2)]
    sigrow = [singles.tile([1, 128], f32, tag=f"sr{p}", name=f"sr{p}") for p in range(2)]
    omsigrow = [singles.tile([1, 128], f32, tag=f"os{p}", name=f"os{p}") for p in range(2)]
    sigpad = singles.tile([128, 32], f32)
    sigtr = singles.tile([128, 32], f32)
    ktSB = singles.tile([40, 128], f32)
    krd = [singles.tile([1, KD, TC], f32, tag=f"kr{p}", name=f"kr{p}") for p in range(2)]

    # ---------------- static init ----------------
    nc.sync.dma_start(whh_lo[:], whht_d[0:128, :])
    nc.sync.dma_start(whh_hi[:], whht_d[128:200, :])
    nc.sync.dma_start(wks_lo[:], wks_d[0:128, :])
    nc.sync.dma_start(wks_hi[:], wks_d[128:225, :])
    nc.vector.memset(ident128[:], 1.0)
    nc.gpsimd.affine_select(ident128[:], ident128[:], [[-1, 128]], OP.is_equal, 0.0,
                            base=0, channel_multiplier=1)
    nc.vector.memset(ident40[:], 1.0)
    nc.gpsimd.affine_select(ident40[:], ident40[:], [[-1, 40]], OP.is_equal, 0.0,
                            base=0, channel_multiplier=1)
    nc.vector.memset(ones40[:], 1.0)
    nc.vector.memset(ones128c[:], 1.0)
    nc.vector.memset(h16[:], 0.0)
    nc.vector.memset(tg[:], 0.0)
    nc.vector.memset(Mt[:], 1e-6)
    nc.vector.memset(rn[:], 1.0)
    nc.vector.memset(rs[:], 1.0)
    nc.vector.memset(sigpad[:], 0.0)
    for p in range(2):
        nc.vector.memset(hidc_b[p][:], 0.0)
        nc.vector.memset(hidc_b[p][96:97, :], 1.0)

    # ---------------- phase A: GXT ----------------
    with tc.tile_pool(name="pha1", bufs=1) as pha1, \
         tc.tile_pool(name="pha", bufs=3) as pha, \
         tc.tile_pool(name="pha_ps", bufs=2, space="PSUM") as pha_ps:
        xT = [pha1.tile([128, T], f32, tag=f"xT{k}", name=f"xT{k}") for k in range(4)]
        for k in range(4):
            nc.sync.dma_start(xT[k][:], x_d[:, 128 * k:128 * (k + 1)].rearrange("t d -> d t"))
        win_sb = pha1.tile([128, 4, F], f32)
        nc.sync.dma_start(win_sb[:], win_d.rearrange("(k p) f -> p k f", p=128))
        binc = pha1.tile([128, 2], f32)
        nc.sync.dma_start(binc[:], bin_d.rearrange("m p -> p m"))
        wat0 = pha1.tile([128, G4P], f32)
        wat1 = pha1.tile([128, G4P], f32)
        wat2 = pha1.tile([2, G4P], f32)
        nc.sync.dma_start(wat0[:], watil_d[0:128, :])
        nc.sync.dma_start(wat1[:], watil_d[128:256, :])
        nc.sync.dma_start(wat2[:], watil_d[256:258, :])

        xys0 = pha1.tile([128, T], f32)
        xys1 = pha1.tile([128, T], f32)
        xys2 = pha1.tile([2, T], f32)
        nc.sync.dma_start(xys2[:], y_d[:])

        # xsT = W_in.T @ x.T  (+ b_in)
        for m in range(2):
            dst = xys0 if m == 0 else xys1
            for n in range(8):
                ps = pha_ps.tile([128, 512], f32, tag="psA")
                for k in range(4):
                    nc.tensor.matmul(ps[:], win_sb[:, k, 128 * m:128 * (m + 1)],
                                     xT[k][:, 512 * n:512 * (n + 1)],
                                     start=(k == 0), stop=(k == 3))
                nc.vector.tensor_scalar(dst[:, 512 * n:512 * (n + 1)], ps[:],
                                        binc[:, m:m + 1], None, OP.add)

        # GXT = Wtil_aug.T-slices @ xysT -> DRAM
        for jj in range(8):
            for n in range(8):
                ps = pha_ps.tile([128, 512], f32, tag="psA")
                nc.tensor.matmul(ps[:], wat0[:, 128 * jj:128 * (jj + 1)],
                                 xys0[:, 512 * n:512 * (n + 1)], start=True, stop=False)
                nc.tensor.matmul(ps[:], wat1[:, 128 * jj:128 * (jj + 1)],
                                 xys1[:, 512 * n:512 * (n + 1)], start=False, stop=False)
                nc.tensor.matmul(ps[:], wat2[:, 128 * jj:128 * (jj + 1)],
                                 xys2[:, 512 * n:512 * (n + 1)], start=False, stop=True)
                stg = pha.tile([128, 512], f32, tag="stgA")
                nc.vector.tensor_copy(stg[:], ps[:])
                nc.sync.dma_start(gxt_d[:, jj, 512 * n:512 * (n + 1)], stg[:])

    # loop-phase PSUM (allocated after phase A pools close: 8 banks exactly)
    psingles = stack.enter_context(tc.tile_pool(name="psingles", bufs=1, space="PSUM"))
    gP = psingles.tile([128, 8], f32)
    delta = psingles.tile([40, 128], f32)
    sn = psingles.tile([128, 2], f32)       # col0 = n2, col1 = s
    se = psingles.tile([1, 1], f32)
    e_row = psingles.tile([1, 128], f32)
    kraw = psingles.tile([128, KD + 1], f32)
    ktr = psingles.tile([40, 128], f32)
    mtp = psingles.tile([NS, KD], f32)
    nc.vector.memset(e_row[:], 0.0)
    nc.vector.memset(e_row[0:1, 0:1], 1.0)

    # ---------------- chunk emitters ----------------
    def emit_B_step(c, gx, s):
        par = c % 2
        ha, hb = hidc_a[par], hidc_b[par]
        # gates = Whh @ h (+ gx added after)
        first = True
        for kc in range(2):
            slab = whh_lo if kc == 0 else whh_hi
            rhs = h16[:, 0:1] if kc == 0 else h16[0:72, 1:2]
            for jj in range(8):
                nc.tensor.matmul(gP[:, jj:jj + 1], slab[:, 128 * jj:128 * (jj + 1)],
                                 rhs, start=first, stop=(kc == 1 and jj == 7))
                first = False
        nc.vector.tensor_add(gsum[:], gP[:], gx[:, :, s])
        nc.scalar.activation(tg[:, 0:8], gsum[:], AF.Tanh)
        nc.vector.tensor_scalar(sg6[:], tg[:, 0:6], 0.5, 0.5, OP.mult, OP.add)
        # c = sig_f*c + sig_i*tanh(gg)   (pair trick: [si|sf] x [tgg|c])
        nc.vector.tensor_mul(gsum[:, 0:4], sg6[:, 0:4], tg[:, 6:10])
        nc.vector.tensor_add(tg[:, 8:10], gsum[:, 0:2], gsum[:, 2:4])
        nc.scalar.activation(thc[:], tg[:, 8:10], AF.Tanh)
        nc.vector.tensor_mul(h16[:], sg6[:, 4:6], thc[:])
        nc.gpsimd.tensor_copy(ha[:, s:s + 1], h16[:, 0:1])
        nc.gpsimd.tensor_copy(hb[0:72, s:s + 1], h16[0:72, 1:2])

    def emit_C(c):
        par = c % 2
        nc.tensor.matmul(kraw[:], hidc_a[par][:], wks_lo[:], start=True, stop=False)
        nc.tensor.matmul(kraw[:], hidc_b[par][:], wks_hi[:], start=False, stop=True)
        nc.scalar.activation(keysc[par][:], kraw[:], AF.Tanh)
        nc.vector.tensor_scalar(sigpad[:, 0:1], keysc[par][:, KD:KD + 1], 0.5, 0.5,
                                OP.mult, OP.add)
        nc.vector.transpose(sigtr[:], sigpad[:])
        for i in range(4):
            nc.gpsimd.tensor_copy(sigrow[par][0:1, 32 * i:32 * (i + 1)],
                                  sigtr[32 * i:32 * i + 1, 0:32])
        nc.vector.tensor_scalar(omsigrow[par][:], sigrow[par][:], -1.0, 1.0,
                                OP.mult, OP.add)
        nc.tensor.transpose(ktr[:], keysc[par][:, 0:KD], ident128[:])
        nc.scalar.activation(knc[par][:], ktr[:], AF.Sign)
        nc.vector.tensor_copy(ktSB[:], ktr[:])
        nc.sync.dma_start(krd[par][:], ktSB[:])

    def emit_D_step(c, s, cold=False):
        par = c % 2
        if True:
            sig_ap = sigrow[par][0:1, s:s + 1]
            omsig_ap = omsigrow[par][0:1, s:s + 1]
            nc.vector.tensor_mul(sigrs[:], sig_ap, rs[:])
            nc.vector.tensor_scalar(ww_row[:], e_row[:], sigrs[:], omsig_ap,
                                    OP.mult, OP.add)
            nc.tensor.matmul(delta[:], krd[par][0:1, :, s], ww_row[:],
                             start=True, stop=True)
            nc.vector.tensor_add(Mt[:], Mt[:], delta[:])
            nc.scalar.activation(Mt2[:], Mt[:], AF.Square)
            nc.tensor.matmul(sn[:, 1:2], Mt[:], knc[par][:, s:s + 1],
                             start=True, stop=False)
            nc.tensor.matmul(sn[:, 0:1], Mt2[:], ones40[:], start=False, stop=True)
            nc.vector.tensor_scalar(n2c[:], sn[:, 0:1], 1e-24, None, OP.max)
            if cold:
                nc.vector.tensor_scalar(qu1[:], n2c.bitcast(u32)[:], 1, None,
                                        OP.logical_shift_right)
                nc.vector.tensor_copy(qf1[:], qu1[:])
                nc.vector.tensor_scalar(qf2[:], qf1[:], -1.0, QUAKE_F, OP.mult, OP.add)
                nc.vector.tensor_copy(qy0[:], qf2[:])
                nc.vector.tensor_copy(rn[:], qy0.bitcast(f32)[:])
                n_iter = 3
            else:
                n_iter = 1
            for _ in range(n_iter):
                nc.vector.tensor_mul(rn2[:], rn[:], rn[:])
                nc.vector.tensor_mul(nt2[:], rn2[:], n2c[:])
                nc.vector.tensor_scalar(nt3[:], nt2[:], -0.5, 1.5, OP.mult, OP.add)
                nc.vector.tensor_mul(rn[:], rn[:], nt3[:])
            nc.vector.tensor_mul(uvec[:], sn[:, 1:2], rn[:])
            # exp(u) = (1+tanh(u/2))/(1-tanh(u/2)) -- keeps the whole program on
            # one ACT table set (no exp<->tanh table reloads)
            nc.scalar.activation(tht[:], uvec[:], AF.Tanh, scale=0.5)
            nc.vector.tensor_scalar(ea[:], tht[:], 1.0, None, OP.add)
            nc.vector.tensor_scalar(eb[:], tht[:], -1.0, 1.0, OP.mult, OP.add)
            nc.vector.reciprocal(eb[:], eb[:])
            nc.vector.tensor_mul(e_col[:], ea[:], eb[:])
            nc.tensor.matmul(se[:], e_col[:], ones128c[:], start=True, stop=True)
            nc.vector.reciprocal(rs[:], se[:])
            nc.tensor.transpose(e_row[:], e_col[:], ident128[:])

    def emit_BD(cB, gx, cD, coldD=False):
        """Memory chunk cD (block) then LSTM chunk cB (block). Per-step
        interleaving of the two chains was measured SLOWER on HW (125 ms vs
        102 ms): alternating ops in the strict-FIFO engine queues couples the
        two serial chains' stalls instead of overlapping them."""
        if cD is not None:
            for s in range(TC):
                emit_D_step(cD, s, coldD)
        for s in range(TC):
            emit_B_step(cB, gx, s)

    # ---------------- prologue + loop + epilogue ----------------
    nc.sync.dma_start(gx_tile[0][:], gxt_d[:, :, 0:TC])
    nc.sync.dma_start(gx_tile[1][:], gxt_d[:, :, TC:2 * TC])
    emit_BD(0, gx_tile[0], None)
    emit_C(0)

    # static section for chunk 1 (runs the only cold D chunk)
    nc.sync.dma_start(gx_tile[0][:], gxt_d[:, :, 2 * TC:3 * TC])
    emit_BD(1, gx_tile[1], 0, coldD=True)
    emit_C(1)

    def emit_section(c_par, iv_expr):
        """Section for B/C chunk with static parity c_par, dynamic index iv_expr."""
        nc.sync.dma_start(gx_tile[(c_par + 1) % 2][:],
                          gxt_d[:, :, ds((iv_expr + 1) * TC, TC)])
        emit_BD(c_par, gx_tile[c_par % 2], c_par - 1)
        emit_C(c_par)

    with tc.For_i(0, 14) as i:
        emit_section(2, 2 * i + 2)
        emit_section(3, 2 * i + 3)

    emit_BD(30, gx_tile[0], 29)
    emit_C(30)
    nc.sync.dma_start(gx_tile[1][:], gxt_d[:, :, 31 * TC:32 * TC])
    emit_BD(31, gx_tile[1], 30)
    emit_C(31)
    for s in range(TC):
        emit_D_step(31, s)

    # output: transpose Mt [40,128] -> [128,40]
    nc.tensor.transpose(mtp[:], Mt[:], ident40[:])
    nc.vector.tensor_copy(m_sb[:], mtp[:])
    nc.sync.dma_start(m_out[:], m_sb[:])

    stack.close()
    return m_out


_CACHE = {}


def _get_program():
    if "nc" not in _CACHE:
        import concourse.bacc as bacc
        import concourse.tile as tile
        nc = bacc.Bacc("TRN2", target_bir_lowering=False, debug=False)
        with tile.TileContext(nc) as tc:
            build(nc, tc)
        nc.compile()
        _CACHE["nc"] = nc
    return _CACHE["nc"]


def kernel(**inputs) -> np.ndarray:
    from concourse import bass_utils
    nc = _get_program()
    in_map = _prep(inputs)
    res = bass_utils.run_bass_kernel_spmd(
        nc, [dict(in_map) for _ in range(N_CORES)], core_ids=list(range(N_CORES))
    )
    return res.results[0]["m_out"]


# revision 23
# speedup vs baseline: 1.5424x; 1.5424x over previous
"""Trainium2 Bass kernel for the MANN network (LSTM scan + memory-write scan).

Self-contained: hardcodes all shapes. kernel(**inputs) takes full numpy inputs
and returns the full [128, 40] final memory matrix.

Structure (single-core program, replicated on 8 cores via SPMD):
  Phase A (batch): GXT[p, jj, t] = gate pre-activations from x/y (PE matmuls -> DRAM)
  Loop over 32 chunks of 128 steps:
    B(c): 128 sequential LSTM steps (tanh-only nonlinearities, fp16 W_hh stationary)
    C(c): chunk keys/sigma/kn batch matmuls
    D(c-1): 128 sequential memory-update steps (PE outer products, Newton rsqrt,
            softmax normalization folded lazily into the next write weights)
"""

import sys

import numpy as np

# concourse (Bass) lives in the TRN RL repo; make it importable regardless of cwd
for _p in ("/opt/trn_rl_repo", "/root/.axon_site/_ro/trn_rl_repo"):
    try:
        import concourse  # noqa: F401
        break
    except ImportError:
        if _p not in sys.path:
            sys.path.insert(0, _p)

T, D, F, H, NS, KD = 4096, 512, 256, 200, 128, 40
TC = 128                  # steps per chunk
NCH = T // TC             # 32 chunks
G4P = 1024                # padded gate vector (4 gates x 256)
QUAKE_F = 1597463007.0    # 0x5f3759df as float
N_CORES = 8


# ---------------------------------------------------------------- host prep --
def _prep(inputs):
    f32 = np.float32
    x = np.ascontiguousarray(inputs["x_train"], f32)
    y = np.ascontiguousarray(inputs["y_train"], f32)
    W_in = np.asarray(inputs["W_in"], f32)
    b_in = np.asarray(inputs["b_in"], f32)
    W_ih = np.asarray(inputs["W_ih"], f32)
    W_hh = np.asarray(inputs["W_hh"], f32)
    b_ih = np.asarray(inputs["b_ih"], f32)
    b_hh = np.asarray(inputs["b_hh"], f32)
    W_k = np.asarray(inputs["W_k"], f32)
    b_k = np.asarray(inputs["b_k"], f32)
    W_s = np.asarray(inputs["W_s"], f32)
    b_s = np.asarray(inputs["b_s"], f32)

    # Gate reorder (i, f, gg, o) -> (i, f, o, gg); sigmoid gates scaled by 0.5
    # (sigmoid(v) = 0.5*tanh(0.5 v)+0.5), pad each gate 200 -> 256 rows.
    gate_src = [0, 1, 3, 2]
    scale = [0.5, 0.5, 0.5, 1.0]
    b_tot = b_ih + b_hh
    Wtil = np.zeros((G4P, F + 2), f32)   # cols 0:256 = x feats, 256 = y, 257 = bias
    Whhp = np.zeros((G4P, H), f32)
    for g in range(4):
        src = gate_src[g]
        rows = slice(256 * g, 256 * g + H)
        Wtil[rows, 0:F + 1] = scale[g] * W_ih[200 * src:200 * src + H, :]
        Wtil[rows, F + 1] = scale[g] * b_tot[200 * src:200 * src + H]
        Whhp[rows, :] = scale[g] * W_hh[200 * src:200 * src + H, :]

    watil_t = np.ascontiguousarray(Wtil.T)                     # [258, 1024] f32
    whhT = np.ascontiguousarray(Whhp.T).astype(np.float16)     # [200, 1024] fp16
    # keys/sigma weights: [201, 41]: rows 0:200 hid, row 200 bias; col 40 scaled W_s
    wks = np.zeros((225, KD + 1), f32)   # 0:200 hid, 200:224 zero pad, 224 bias
    wks[0:H, 0:KD] = W_k
    wks[224, 0:KD] = b_k
    wks[0:H, KD] = 0.5 * W_s[:, 0]
    wks[224, KD] = 0.5 * b_s[0]
    wks = wks.astype(np.float16)

    ysh1 = np.zeros((2, T), f32)          # row0 = y_shift, row1 = ones
    ysh1[0, 1:] = y[:-1, 0]
    ysh1[1, :] = 1.0
    return {
        "x_train": x,
        "ysh1": ysh1,
        "watil_t": watil_t,
        "whht": whhT,
        "wks": wks,
        "w_in": np.ascontiguousarray(W_in),
        "b_in": np.ascontiguousarray(b_in.reshape(2, 128)),   # [m, p] -> load as [128,2] via rearrange
    }


# ------------------------------------------------------------- bass program --
def build(nc, tc):
    import concourse.bass as bass
    from concourse import mybir
    from concourse.bass import ds

    f32 = mybir.dt.float32
    f16 = mybir.dt.float16
    u32 = mybir.dt.uint32
    AF = mybir.ActivationFunctionType
    OP = mybir.AluOpType
    X = mybir.AxisListType.X

    x_d = nc.dram_tensor("x_train", [T, D], f32, kind="ExternalInput")
    y_d = nc.dram_tensor("ysh1", [2, T], f32, kind="ExternalInput")
    watil_d = nc.dram_tensor("watil_t", [F + 2, G4P], f32, kind="ExternalInput")
    whht_d = nc.dram_tensor("whht", [H, G4P], f16, kind="ExternalInput")
    wks_d = nc.dram_tensor("wks", [225, KD + 1], f16, kind="ExternalInput")
    win_d = nc.dram_tensor("w_in", [D, F], f32, kind="ExternalInput")
    bin_d = nc.dram_tensor("b_in", [2, 128], f32, kind="ExternalInput")
    m_out = nc.dram_tensor("m_out", [NS, KD], f32, kind="ExternalOutput")
    gxt_d = nc.dram_tensor("gxt", [128, 8, T + TC], f32)  # internal scratch

    from contextlib import ExitStack
    stack = ExitStack()

    singles = stack.enter_context(tc.tile_pool(name="singles", bufs=1))

    # ---------------- persistent loop tiles ----------------
    whh_lo = singles.tile([128, G4P], f16)
    whh_hi = singles.tile([72, G4P], f16)
    wks_lo = singles.tile([128, KD + 1], f16)
    wks_hi = singles.tile([97, KD + 1], f16)
    ident128 = singles.tile([128, 128], f32)
    ident40 = singles.tile([40, 40], f32)
    ones40 = singles.tile([40, 1], f32)
    ones128c = singles.tile([128, 1], f32)

    h16 = singles.tile([128, 2], f16)       # hidden state (col0 = h[0:128], col1 = h[128:200]+pad)
    tg = singles.tile([128, 10], f32)       # cols 0:8 tanh(gates); cols 8:10 = c state
    sg6 = singles.tile([128, 6], f32)       # sigmoid(i,f,o)
    gsum = singles.tile([128, 8], f32)      # gates pre-activation (gx + Whh h)
    thc = singles.tile([128, 2], f32)

    Mt = singles.tile([40, 128], f32)       # memory, transposed [key, slot]
    Mt2 = singles.tile([40, 128], f32)
    rn = singles.tile([128, 1], f32)        # rsqrt(row-norm^2) newton state
    rn2 = singles.tile([128, 1], f32)
    n2c = singles.tile([128, 1], f32)
    nt2 = singles.tile([128, 1], f32)
    nt3 = singles.tile([128, 1], f32)
    qu1 = singles.tile([128, 1], u32)
    qf1 = singles.tile([128, 1], f32)
    qf2 = singles.tile([128, 1], f32)
    qy0 = singles.tile([128, 1], u32)
    uvec = singles.tile([128, 1], f32)
    tht = singles.tile([128, 1], f32)
    ea = singles.tile([128, 1], f32)
    eb = singles.tile([128, 1], f32)
    e_col = singles.tile([128, 1], f32)
    ww_row = singles.tile([1, 128], f32)
    rs = singles.tile([1, 1], f32)
    sigrs = singles.tile([1, 1], f32)
    m_sb = singles.tile([NS, KD], f32)

    # ping-pong chunk tiles
    gx_tile = [singles.tile([128, 8, TC], f32, tag=f"gx{p}", name=f"gx{p}") for p in range(2)]
    hidc_a = [singles.tile([128, TC], f16, tag=f"ha{p}", name=f"ha{p}") for p in range(2)]
    hidc_b = [singles.tile([97, TC], f16, tag=f"hb{p}", name=f"hb{p}") for p in range(2)]
    keysc = [singles.tile([128, KD + 1], f32, tag=f"kc{p}", name=f"kc{p}") for p in range(2)]
    knc = [singles.tile([40, 128], f32, tag=f"kn{p}", name=f"kn{p}") for p in range(2)]
    sigrow = [singles.tile([1, 128], f32, tag=f"sr{p}", name=f"sr{p}") for p in range(2)]
    omsigrow = [singles.tile([1, 128], f32, tag=f"os{p}", name=f"os{p}") for p in range(2)]
    sigpad = singles.tile([128, 32], f32)
    sigtr = singles.tile([128, 32], f32)
    ktSB = singles.tile([40, 128], f32)
    krd = [singles.tile([1, KD, TC], f32, tag=f"kr{p}", name=f"kr{p}") for p in range(2)]

    # ---------------- static init ----------------
    nc.sync.dma_start(whh_lo[:], whht_d[0:128, :])
    nc.sync.dma_start(whh_hi[:], whht_d[128:200, :])
    nc.sync.dma_start(wks_lo[:], wks_d[0:128, :])
    nc.sync.dma_start(wks_hi[:], wks_d[128:225, :])
    nc.vector.memset(ident128[:], 1.0)
    nc.gpsimd.affine_select(ident128[:], ident128[:], [[-1, 128]], OP.is_equal, 0.0,
                            base=0, channel_multiplier=1)
    nc.vector.memset(ident40[:], 1.0)
    nc.gpsimd.affine_select(ident40[:], ident40[:], [[-1, 40]], OP.is_equal, 0.0,
                            base=0, channel_multiplier=1)
    nc.vector.memset(ones40[:], 1.0)
    nc.vector.memset(ones128c[:], 1.0)
    nc.vector.memset(h16[:], 0.0)
    nc.vector.memset(tg[:], 0.0)
    nc.vector.memset(Mt[:], 1e-6)
    nc.vector.memset(rn[:], 1.0)
    nc.vector.memset(rs[:], 1.0)
    nc.vector.memset(sigpad[:], 0.0)
    for p in range(2):
        nc.vector.memset(hidc_b[p][:], 0.0)
        nc.vector.memset(hidc_b[p][96:97, :], 1.0)

    # ---------------- phase A: GXT ----------------
    with tc.tile_pool(name="pha1", bufs=1) as pha1, \
         tc.tile_pool(name="pha", bufs=3) as pha, \
         tc.tile_pool(name="pha_ps", bufs=2, space="PSUM") as pha_ps:
        xT = [pha1.tile([128, T], f32, tag=f"xT{k}", name=f"xT{k}") for k in range(4)]
        for k in range(4):
            nc.sync.dma_start(xT[k][:], x_d[:, 128 * k:128 * (k + 1)].rearrange("t d -> d t"))
        win_sb = pha1.tile([128, 4, F], f32)
        nc.sync.dma_start(win_sb[:], win_d.rearrange("(k p) f -> p k f", p=128))
        binc = pha1.tile([128, 2], f32)
        nc.sync.dma_start(binc[:], bin_d.rearrange("m p -> p m"))
        wat0 = pha1.tile([128, G4P], f32)
        wat1 = pha1.tile([128, G4P], f32)
        wat2 = pha1.tile([2, G4P], f32)
        nc.sync.dma_start(wat0[:], watil_d[0:128, :])
        nc.sync.dma_start(wat1[:], watil_d[128:256, :])
        nc.sync.dma_start(wat2[:], watil_d[256:258, :])

        xys0 = pha1.tile([128, T], f32)
        xys1 = pha1.tile([128, T], f32)
        xys2 = pha1.tile([2, T], f32)
        nc.sync.dma_start(xys2[:], y_d[:])

        # xsT = W_in.T @ x.T  (+ b_in)
        for m in range(2):
            dst = xys0 if m == 0 else xys1
            for n in range(8):
                ps = pha_ps.tile([128, 512], f32, tag="psA")
                for k in range(4):
                    nc.tensor.matmul(ps[:], win_sb[:, k, 128 * m:128 * (m + 1)],
                                     xT[k][:, 512 * n:512 * (n + 1)],
                                     start=(k == 0), stop=(k == 3))
                nc.vector.tensor_scalar(dst[:, 512 * n:512 * (n + 1)], ps[:],
                                        binc[:, m:m + 1], None, OP.add)

        # GXT = Wtil_aug.T-slices @ xysT -> DRAM
        for jj in range(8):
            for n in range(8):
                ps = pha_ps.tile([128, 512], f32, tag="psA")
                nc.tensor.matmul(ps[:], wat0[:, 128 * jj:128 * (jj + 1)],
                                 xys0[:, 512 * n:512 * (n + 1)], start=True, stop=False)
                nc.tensor.matmul(ps[:], wat1[:, 128 * jj:128 * (jj + 1)],
                                 xys1[:, 512 * n:512 * (n + 1)], start=False, stop=False)
                nc.tensor.matmul(ps[:], wat2[:, 128 * jj:128 * (jj + 1)],
                                 xys2[:, 512 * n:512 * (n + 1)], start=False, stop=True)
                stg = pha.tile([128, 512], f32, tag="stgA")
                nc.vector.tensor_copy(stg[:], ps[:])
                nc.sync.dma_start(gxt_d[:, jj, 512 * n:512 * (n + 1)], stg[:])

    # loop-phase PSUM (allocated after phase A pools close: 8 banks exactly)
    psingles = stack.enter_context(tc.tile_pool(name="psingles", bufs=1, space="PSUM"))
    gP = psingles.tile([128, 8], f32)
    delta = psingles.tile([40, 128], f32)
    sn = psingles.tile([128, 2], f32)       # col0 = n2, col1 = s
    se = psingles.tile([1, 1], f32)
    e_row = psingles.tile([1, 128], f32)
    kraw = psingles.tile([128, KD + 1], f32)
    ktr = psingles.tile([40, 128], f32)
    mtp = psingles.tile([NS, KD], f32)
    nc.vector.memset(e_row[:], 0.0)
    nc.vector.memset(e_row[0:1, 0:1], 1.0)

    # ---------------- chunk emitters ----------------
    def emit_B_step(c, gx, s):
        par = c % 2
        ha, hb = hidc_a[par], hidc_b[par]
        # gates = gx + Whh @ h. The gx term enters PSUM via an identity matmul
        # (depends only on the prefetched gx tile, so it runs ahead of the
        # h-dependent matmuls and removes a DVE add + 2 sync hops per step).
        nc.tensor.matmul(gP[:], ident128[:], gx[:, :, s], start=True, stop=False)
        for kc in range(2):
            slab = whh_lo if kc == 0 else whh_hi
            rhs = h16[:, 0:1] if kc == 0 else h16[0:72, 1:2]
            for jj in range(8):
                nc.tensor.matmul(gP[:, jj:jj + 1], slab[:, 128 * jj:128 * (jj + 1)],
                                 rhs, start=False, stop=(kc == 1 and jj == 7))
        nc.scalar.activation(tg[:, 0:8], gP[:], AF.Tanh)
        nc.vector.tensor_scalar(sg6[:], tg[:, 0:6], 0.5, 0.5, OP.mult, OP.add)
        # c = sig_f*c + sig_i*tanh(gg)   (pair trick: [si|sf] x [tgg|c])
        nc.vector.tensor_mul(gsum[:, 0:4], sg6[:, 0:4], tg[:, 6:10])
        nc.vector.tensor_add(tg[:, 8:10], gsum[:, 0:2], gsum[:, 2:4])
        nc.scalar.activation(thc[:], tg[:, 8:10], AF.Tanh)
        nc.vector.tensor_mul(h16[:], sg6[:, 4:6], thc[:])
        nc.gpsimd.tensor_copy(ha[:, s:s + 1], h16[:, 0:1])
        nc.gpsimd.tensor_copy(hb[0:72, s:s + 1], h16[0:72, 1:2])

    def emit_C(c):
        par = c % 2
        nc.tensor.matmul(kraw[:], hidc_a[par][:], wks_lo[:], start=True, stop=False)
        nc.tensor.matmul(kraw[:], hidc_b[par][:], wks_hi[:], start=False, stop=True)
        nc.scalar.activation(keysc[par][:], kraw[:], AF.Tanh)
        nc.vector.tensor_scalar(sigpad[:, 0:1], keysc[par][:, KD:KD + 1], 0.5, 0.5,
                                OP.mult, OP.add)
        nc.vector.transpose(sigtr[:], sigpad[:])
        for i in range(4):
            nc.gpsimd.tensor_copy(sigrow[par][0:1, 32 * i:32 * (i + 1)],
                                  sigtr[32 * i:32 * i + 1, 0:32])
        nc.vector.tensor_scalar(omsigrow[par][:], sigrow[par][:], -1.0, 1.0,
                                OP.mult, OP.add)
        nc.tensor.transpose(ktr[:], keysc[par][:, 0:KD], ident128[:])
        nc.scalar.activation(knc[par][:], ktr[:], AF.Sign)
        nc.vector.tensor_copy(ktSB[:], ktr[:])
        nc.sync.dma_start(krd[par][:], ktSB[:])

    def emit_D_step(c, s, cold=False):
        par = c % 2
        if True:
            sig_ap = sigrow[par][0:1, s:s + 1]
            omsig_ap = omsigrow[par][0:1, s:s + 1]
            nc.vector.tensor_mul(sigrs[:], sig_ap, rs[:])
            nc.vector.tensor_scalar(ww_row[:], e_row[:], sigrs[:], omsig_ap,
                                    OP.mult, OP.add)
            nc.tensor.matmul(delta[:], krd[par][0:1, :, s], ww_row[:],
                             start=True, stop=True)
            nc.vector.tensor_add(Mt[:], Mt[:], delta[:])
            nc.scalar.activation(Mt2[:], Mt[:], AF.Square)
            nc.tensor.matmul(sn[:, 1:2], Mt[:], knc[par][:, s:s + 1],
                             start=True, stop=False)
            nc.tensor.matmul(sn[:, 0:1], Mt2[:], ones40[:], start=False, stop=True)
            nc.vector.tensor_scalar(n2c[:], sn[:, 0:1], 1e-24, None, OP.max)
            if cold:
                nc.vector.tensor_scalar(qu1[:], n2c.bitcast(u32)[:], 1, None,
                                        OP.logical_shift_right)
                nc.vector.tensor_copy(qf1[:], qu1[:])
                nc.vector.tensor_scalar(qf2[:], qf1[:], -1.0, QUAKE_F, OP.mult, OP.add)
                nc.vector.tensor_copy(qy0[:], qf2[:])
                nc.vector.tensor_copy(rn[:], qy0.bitcast(f32)[:])
                n_iter = 3
            else:
                n_iter = 1
            for _ in range(n_iter):
                nc.vector.tensor_mul(rn2[:], rn[:], rn[:])
                nc.vector.tensor_mul(nt2[:], rn2[:], n2c[:])
                nc.vector.tensor_scalar(nt3[:], nt2[:], -0.5, 1.5, OP.mult, OP.add)
                nc.vector.tensor_mul(rn[:], rn[:], nt3[:])
            nc.vector.tensor_mul(uvec[:], sn[:, 1:2], rn[:])
            # exp(u) = (1+tanh(u/2))/(1-tanh(u/2)) -- keeps the whole program on
            # one ACT table set (no exp<->tanh table reloads)
            nc.scalar.activation(tht[:], uvec[:], AF.Tanh, scale=0.5)
            nc.vector.tensor_scalar(ea[:], tht[:], 1.0, None, OP.add)
            nc.vector.tensor_scalar(eb[:], tht[:], -1.0, 1.0, OP.mult, OP.add)
            nc.vector.reciprocal(eb[:], eb[:])
            nc.vector.tensor_mul(e_col[:], ea[:], eb[:])
            nc.tensor.matmul(se[:], e_col[:], ones128c[:], start=True, stop=True)
            nc.vector.reciprocal(rs[:], se[:])
            nc.tensor.transpose(e_row[:], e_col[:], ident128[:])

    def emit_BD(cB, gx, cD, coldD=False):
        """Memory chunk cD (block) then LSTM chunk cB (block). Per-step
        interleaving of the two chains was measured SLOWER on HW (125 ms vs
        102 ms): alternating ops in the strict-FIFO engine queues couples the
        two serial chains' stalls instead of overlapping them."""
        if cD is not None:
            for s in range(TC):
                emit_D_step(cD, s, coldD)
        for s in range(TC):
            emit_B_step(cB, gx, s)

    # ---------------- prologue + loop + epilogue ----------------
    nc.sync.dma_start(gx_tile[0][:], gxt_d[:, :, 0:TC])
    nc.sync.dma_start(gx_tile[1][:], gxt_d[:, :, TC:2 * TC])
    emit_BD(0, gx_tile[0], None)
    emit_C(0)

    # static section for chunk 1 (runs the only cold D chunk)
    nc.sync.dma_start(gx_tile[0][:], gxt_d[:, :, 2 * TC:3 * TC])
    emit_BD(1, gx_tile[1], 0, coldD=True)
    emit_C(1)

    def emit_section(c_par, iv_expr):
        """Section for B/C chunk with static parity c_par, dynamic index iv_expr."""
        nc.sync.dma_start(gx_tile[(c_par + 1) % 2][:],
                          gxt_d[:, :, ds((iv_expr + 1) * TC, TC)])
        emit_BD(c_par, gx_tile[c_par % 2], c_par - 1)
        emit_C(c_par)

    with tc.For_i(0, 14) as i:
        emit_section(2, 2 * i + 2)
        emit_section(3, 2 * i + 3)

    emit_BD(30, gx_tile[0], 29)
    emit_C(30)
    nc.sync.dma_start(gx_tile[1][:], gxt_d[:, :, 31 * TC:32 * TC])
    emit_BD(31, gx_tile[1], 30)
    emit_C(31)
    for s in range(TC):
        emit_D_step(31, s)

    # output: transpose Mt [40,128] -> [128,40]
    nc.tensor.transpose(mtp[:], Mt[:], ident40[:])
    nc.vector.tensor_copy(m_sb[:], mtp[:])
    nc.sync.dma_start(m_out[:], m_sb[:])

    stack.close()
    return m_out


_CACHE = {}


def _get_program():
    if "nc" not in _CACHE:
        import concourse.bacc as bacc
        import concourse.tile as tile
        nc = bacc.Bacc("TRN2", target_bir_lowering=False, debug=False)
        with tile.TileContext(nc) as tc:
            build(nc, tc)
        nc.compile()
        _CACHE["nc"] = nc
    return _CACHE["nc"]


def kernel(**inputs) -> np.ndarray:
    from concourse import bass_utils
    nc = _get_program()
    in_map = _prep(inputs)
    res = bass_utils.run_bass_kernel_spmd(
        nc, [dict(in_map) for _ in range(N_CORES)], core_ids=list(range(N_CORES))
    )
    return res.results[0]["m_out"]
